# revision 3
# baseline (speedup 1.0000x reference)
"""Trainium2 Bass kernel for nn_CrossAttentionBlock.

Math note: the reference's attention has a length-1 key axis, so
softmax(attn, axis=-1) == 1.0 exactly and the attention output equals v
broadcast over the HW query axis.  The GroupNorm -> Wq -> q@k path is
therefore mathematically dead.  The exact output is

    out[b, c, h, w] = x[b, c, h, w] + y[b, c]
    y[b]            = Wout @ v[b] + bout
    v[b]            = Wkv[C:2C, :] @ context[b] + bkv[C:2C]

Sharding: pure data parallel over batch B=32 -> 4 batches per core on 8
cores; the small weights are replicated.  Per core the kernel computes
the tiny (256x512 / 256x256) matmuls on the TensorEngine and then
streams the 16.8 MB x-shard through SBUF adding the per-(b,c) scalar.
"""

import numpy as np

import concourse.bass as bass
import concourse.mybir as mybir
import concourse.tile as tile
from concourse import bacc
from concourse.bass_utils import run_bass_kernel_spmd
from concourse.masks import make_identity

N_CORES = 8
B = 32
C = 256
HW = 64 * 64
CTX = 512
B_LOC = B // N_CORES                # 4 batches per core
ROWS = B_LOC * C                    # 1024 rows of the flattened x-shard
ROW_TILES = ROWS // 128             # 8 streaming tiles of [128, HW]
FP32 = mybir.dt.float32

_cache: dict = {}


def _build_nc() -> bass.Bass:
    nc = bacc.Bacc("TRN2", target_bir_lowering=False, debug=False)

    xs = nc.dram_tensor("xs", [ROWS, HW], FP32, kind="ExternalInput")
    ctxs = nc.dram_tensor("ctxs", [B_LOC, CTX], FP32, kind="ExternalInput")
    wkv_v = nc.dram_tensor("wkv_v", [C, CTX], FP32, kind="ExternalInput")
    bkv_v = nc.dram_tensor("bkv_v", [C], FP32, kind="ExternalInput")
    wout = nc.dram_tensor("wout", [C, C], FP32, kind="ExternalInput")
    bout = nc.dram_tensor("bout", [C], FP32, kind="ExternalInput")
    out = nc.dram_tensor("out", [ROWS, HW], FP32, kind="ExternalOutput")

    with tile.TileContext(nc) as tc:
        with (
            tc.tile_pool(name="consts", bufs=1) as consts,
            tc.tile_pool(name="psum_t", bufs=4, space="PSUM") as psum_t,
            tc.tile_pool(name="psum_mm", bufs=2, space="PSUM") as psum_mm,
            tc.tile_pool(name="stream", bufs=6) as stream,
        ):
            # ---- small-weight pipeline: y[c, b] on partitions c ----
            identity = consts.tile([128, 128], FP32)
            make_identity(nc, identity)

            # ctx^T: [CTX, B_LOC] as 4 k-chunks of [128, B_LOC]
            ctx_pad = consts.tile([128, CTX], FP32)
            nc.gpsimd.memset(ctx_pad[:], 0.0)
            nc.sync.dma_start(out=ctx_pad[:B_LOC, :], in_=ctxs[:, :])
            ctxT = consts.tile([128, CTX // 128, B_LOC], FP32)
            for kc in range(CTX // 128):
                pt = psum_t.tile([128, 128], FP32)
                nc.tensor.transpose(pt[:], ctx_pad[:, kc * 128:(kc + 1) * 128], identity[:])
                nc.vector.tensor_copy(out=ctxT[:, kc, :], in_=pt[:, :B_LOC])

            # Wkv_v^T: [CTX, C] as 4 k-chunks of [128, C]
            wv_sb = consts.tile([128, C // 128, CTX], FP32)
            nc.sync.dma_start(out=wv_sb[:], in_=wkv_v.rearrange("(o p) k -> p o k", p=128))
            wkvT = consts.tile([128, CTX // 128, C], FP32)
            for kc in range(CTX // 128):
                for cc in range(C // 128):
                    pt = psum_t.tile([128, 128], FP32)
                    nc.tensor.transpose(
                        pt[:], wv_sb[:, cc, kc * 128:(kc + 1) * 128], identity[:]
                    )
                    nc.vector.tensor_copy(
                        out=wkvT[:, kc, cc * 128:(cc + 1) * 128], in_=pt[:]
                    )

            # Wout^T: [C, C] as 2 c-chunks of [128, C]
            wo_sb = consts.tile([128, C // 128, C], FP32)
            nc.sync.dma_start(out=wo_sb[:], in_=wout.rearrange("(o p) c -> p o c", p=128))
            woT = consts.tile([128, C // 128, C], FP32)
            for r in range(C // 128):
                for s in range(C // 128):
                    pt = psum_t.tile([128, 128], FP32)
                    nc.tensor.transpose(
                        pt[:], wo_sb[:, r, s * 128:(s + 1) * 128], identity[:]
                    )
                    nc.vector.tensor_copy(
                        out=woT[:, s, r * 128:(r + 1) * 128], in_=pt[:]
                    )

            # biases striped to partitions: [128, 2] (column = 128-row chunk)
            bkv_sb = consts.tile([128, C // 128], FP32)
            nc.sync.dma_start(out=bkv_sb[:], in_=bkv_v.rearrange("(o p) -> p o", p=128))
            bout_sb = consts.tile([128, C // 128], FP32)
            nc.sync.dma_start(out=bout_sb[:], in_=bout.rearrange("(o p) -> p o", p=128))

            # v[c, b] = Wkv_v @ ctx^T + bkv_v
            v_sb = consts.tile([128, C // 128, B_LOC], FP32)
            for cc in range(C // 128):
                pv = psum_mm.tile([128, B_LOC], FP32)
                for kc in range(CTX // 128):
                    nc.tensor.matmul(
                        pv[:],
                        wkvT[:, kc, cc * 128:(cc + 1) * 128],
                        ctxT[:, kc, :],
                        start=(kc == 0),
                        stop=(kc == CTX // 128 - 1),
                    )
                nc.vector.tensor_tensor(
                    v_sb[:, cc, :],
                    pv[:],
                    bkv_sb[:, cc:cc + 1].to_broadcast([128, B_LOC]),
                    mybir.AluOpType.add,
                )

            # y[o, b] = Wout @ v + bout, laid out as yb[p, oc, b]
            yb = consts.tile([128, C // 128, B_LOC], FP32)
            for oc in range(C // 128):
                py = psum_mm.tile([128, B_LOC], FP32)
                for cc in range(C // 128):
                    nc.tensor.matmul(
                        py[:],
                        woT[:, cc, oc * 128:(oc + 1) * 128],
                        v_sb[:, cc, :],
                        start=(cc == 0),
                        stop=(cc == C // 128 - 1),
                    )
                nc.vector.tensor_tensor(
                    yb[:, oc, :],
                    py[:],
                    bout_sb[:, oc:oc + 1].to_broadcast([128, B_LOC]),
                    mybir.AluOpType.add,
                )

            # ---- stream x through SBUF: out = x + y[b, c] ----
            for t in range(ROW_TILES):
                b, oc = t // (C // 128), t % (C // 128)
                xt = stream.tile([128, HW], FP32)
                nc.sync.dma_start(out=xt[:], in_=xs[t * 128:(t + 1) * 128, :])
                nc.vector.tensor_tensor(
                    xt[:],
                    xt[:],
                    yb[:, oc, b:b + 1].to_broadcast([128, HW]),
                    mybir.AluOpType.add,
                )
                # stores on the ACT HWDGE ring so they don't queue behind loads
                nc.scalar.dma_start(out=out[t * 128:(t + 1) * 128, :], in_=xt[:])

    nc.finalize()
    return nc


def kernel(x, context, gn_w=None, gn_b=None, Wq=None, bq=None, Wkv=None,
           bkv=None, Wout=None, bout=None, _trace=False):
    # gn_w/gn_b/Wq/bq and the k-half of Wkv/bkv are mathematically dead
    # (softmax over a length-1 axis is exactly 1), so they are unused.
    x = np.ascontiguousarray(np.asarray(x, dtype=np.float32))
    context = np.ascontiguousarray(np.asarray(context, dtype=np.float32))
    Wkv = np.asarray(Wkv, dtype=np.float32)
    bkv = np.asarray(bkv, dtype=np.float32)
    wkv_v = np.ascontiguousarray(Wkv[C:2 * C])
    bkv_v = np.ascontiguousarray(bkv[C:2 * C])
    wout = np.ascontiguousarray(np.asarray(Wout, dtype=np.float32))
    bout_np = np.ascontiguousarray(np.asarray(bout, dtype=np.float32))

    if "nc" not in _cache:
        _cache["nc"] = _build_nc()
    nc = _cache["nc"]

    in_maps = []
    for c in range(N_CORES):
        xs = x[c * B_LOC:(c + 1) * B_LOC].reshape(ROWS, HW)
        in_maps.append({
            "xs": np.ascontiguousarray(xs),
            "ctxs": np.ascontiguousarray(context[c * B_LOC:(c + 1) * B_LOC]),
            "wkv_v": wkv_v,
            "bkv_v": bkv_v,
            "wout": wout,
            "bout": bout_np,
        })

    res = run_bass_kernel_spmd(nc, in_maps, core_ids=list(range(N_CORES)),
                               trace=_trace)
    kernel.last_result = res
    out = np.concatenate(
        [r["out"].reshape(B_LOC, C, 64, 64) for r in res.results], axis=0
    )
    return out
